# revision 1
# baseline (speedup 1.0000x reference)
"""Trainium2 Bass kernel for a 2-layer GAT (DGL-style) over a random graph.

Strategy (edge-parallel, dst-owner sharding, 8 NeuronCores):
  * Nodes are sorted by in-degree and relabeled into "positions"; 128-node
    blocks of similar degree are dealt snake-wise to the 8 cores so per-core
    work is balanced.  Core c owns positions [c*NBC*128, (c+1)*NBC*128).
  * Every core redundantly computes the full feature table
    table[q] = [feat(q) | el(q) | er(q)]  (feat = h @ W.T, el = feat@al,
    er = feat@ar, fused into one matmul with W_ext), so no collective is
    needed to distribute it.
  * Edges are laid out in ELL format per dst block: an int32 index grid
    [128, 1+K] holds (own position | src positions...), padded with a
    sentinel row whose el = -200 (=> zero attention weight).  One indirect
    DMA per ELL column gathers table rows straight into the ELL layout.
  * Softmax over incoming edges + weighted message sum are computed in the
    node-major (ELL) orientation with a handful of DVE/ACT ops per block.
  * Layer-1 outputs x are produced transposed (PE transpose) and AllGathered
    (the only collective), then layer 2 repeats the same structure.
Host side does only index/permutation planning + final unpermute.
"""

import math

import numpy as np

import concourse.bass as bass
import concourse.bacc as bacc
import concourse.tile as tile
from concourse import mybir
from concourse.bass_utils import run_bass_kernel_spmd
from concourse.masks import make_identity

P = 128
NCORES = 8
DIN = 128
DH = 32  # hidden dim == out dim
DEXT = DH + 2  # feat | el | er
NEG = 0.2
SENT_EL = -200.0
F32 = mybir.dt.float32
I32 = mybir.dt.int32

GBLK = 6  # blocks per gather group (SBUF staging granularity)
FEATG = 4  # feat-phase blocks per group


def _plan(src, dst, n_nodes):
    """Host-side planning: node permutation, ELL index grids, groupings."""
    E = src.shape[0]
    deg = np.bincount(dst, minlength=n_nodes)
    order = np.argsort(-deg, kind="stable")  # nodes by desc in-degree

    NB = math.ceil(n_nodes / P)
    NBC = math.ceil(NB / NCORES)  # blocks per core
    NBT = NBC * NCORES
    NPOS = NBT * P
    SENT = NPOS
    VROWS = NPOS + 1

    node_at_s = np.full(NPOS, -1, np.int64)
    node_at_s[:n_nodes] = order

    sb = np.arange(NBT)
    r = sb // NCORES
    mcol = sb % NCORES
    core_of_sb = np.where(r % 2 == 0, mcol, NCORES - 1 - mcol)

    s = np.arange(NPOS)
    sb_s = s // P
    q_of_s = (core_of_sb[sb_s] * NBC + r[sb_s]) * P + (s % P)

    vmask = node_at_s >= 0
    q_of_node = np.full(n_nodes, -1, np.int64)
    q_of_node[node_at_s[vmask]] = q_of_s[vmask]
    node_at_q = np.full(NPOS, -1, np.int64)
    node_at_q[q_of_s] = node_at_s

    deg_sorted = deg[order]
    first_idx = np.arange(NBC) * NCORES * P
    K_sh = np.zeros(NBC, np.int64)
    in_range = first_idx < n_nodes
    K_sh[in_range] = deg_sorted[first_idx[in_range]]
    K_sh = np.maximum(K_sh, 1)
    width = 1 + K_sh
    cstart = np.concatenate([[0], np.cumsum(width)]).astype(np.int64)
    SW = int(cstart[-1])

    sidx = np.full((NCORES, P, SW), SENT, np.int32)
    qown = (
        (np.arange(NCORES)[:, None, None] * NBC + np.arange(NBC)[None, :, None]) * P
        + np.arange(P)[None, None, :]
    )
    sidx[:, np.arange(P)[:, None], cstart[:-1][None, :]] = qown.transpose(0, 2, 1)

    qs = q_of_node[src]
    qd = q_of_node[dst]
    eorder = np.lexsort((qs, qd))
    qd_s = qd[eorder]
    qs_s = qs[eorder]
    first_of_val = np.searchsorted(qd_s, qd_s, side="left")
    rank = np.arange(E) - first_of_val
    c_e = qd_s // (NBC * P)
    i_e = (qd_s // P) % NBC
    p_e = qd_s % P
    assert (rank < K_sh[i_e]).all(), "ELL rank exceeded block K"
    col_e = cstart[i_e] + 1 + rank
    sidx[c_e, p_e, col_e] = qs_s

    groups = []
    i0 = 0
    while i0 < NBC:
        i1 = min(i0 + GBLK, NBC)
        groups.append((i0, i1, int(cstart[i0]), int(cstart[i1] - cstart[i0])))
        i0 = i1
    GC = max(g[3] for g in groups)

    return dict(
        NB=NB, NBC=NBC, NBT=NBT, NPOS=NPOS, SENT=SENT, VROWS=VROWS,
        K_sh=K_sh, cstart=cstart, SW=SW, sidx=sidx, groups=groups, GC=GC,
        q_of_s=q_of_s, vmask=vmask, node_at_s=node_at_s, node_at_q=node_at_q,
        q_of_node=q_of_node,
    )


def _build_nc(plan):
    NBC = plan["NBC"]
    NBT = plan["NBT"]
    NPOS = plan["NPOS"]
    SENT = plan["SENT"]
    VROWS = plan["VROWS"]
    K_sh = plan["K_sh"]
    cstart = plan["cstart"]
    SW = plan["SW"]
    groups = plan["groups"]
    GC = plan["GC"]
    NOWN = NBC * P

    nc = bacc.Bacc(None, num_devices=NCORES)

    ht_in = nc.dram_tensor("ht", [DIN, NPOS], F32, kind="ExternalInput")
    sidx_in = nc.dram_tensor("sidx", [P, SW], I32, kind="ExternalInput")
    w1_in = nc.dram_tensor("w1ext", [DIN, DEXT], F32, kind="ExternalInput")
    w2_in = nc.dram_tensor("w2ext", [DH, DEXT], F32, kind="ExternalInput")
    b1_in = nc.dram_tensor("b1c", [DH, 1], F32, kind="ExternalInput")
    b2_in = nc.dram_tensor("b2t", [P, DH], F32, kind="ExternalInput")
    out_ext = nc.dram_tensor("out", [NOWN, DH], F32, kind="ExternalOutput")

    table1 = nc.dram_tensor("table1", [VROWS, DEXT], F32, kind="Internal")
    table2 = nc.dram_tensor("table2", [VROWS, DEXT], F32, kind="Internal")
    xt_own = nc.dram_tensor("xt_own", [DH, NOWN], F32, kind="Internal")
    xt_full = nc.dram_tensor(
        "xt_full", [NCORES, DH, NOWN], F32, kind="Internal", addr_space="Shared"
    )

    with tile.TileContext(nc) as tc:
        with (
            tc.tile_pool(name="const", bufs=1) as constp,
            tc.tile_pool(name="sidxp", bufs=1) as sidxp,
            tc.tile_pool(name="feat_in", bufs=3) as featin,
            tc.tile_pool(name="feat_out", bufs=3) as featout,
            tc.tile_pool(name="gath", bufs=2) as gathp,
            tc.tile_pool(name="agg", bufs=2) as aggp,
            tc.tile_pool(name="small", bufs=4) as smallp,
            tc.tile_pool(name="xt", bufs=1) as xtp,
            tc.tile_pool(name="psum", bufs=4, space="PSUM") as psum,
            tc.tile_pool(name="psum_t", bufs=2, space="PSUM") as psum_t,
        ):
            w1_s = constp.tile([DIN, DEXT], F32)
            nc.sync.dma_start(out=w1_s[:], in_=w1_in[:])
            w2_s = constp.tile([DH, DEXT], F32)
            nc.sync.dma_start(out=w2_s[:], in_=w2_in[:])
            b1_s = constp.tile([DH, 1], F32)
            nc.sync.dma_start(out=b1_s[:], in_=b1_in[:])
            b2_s = constp.tile([P, DH], F32)
            nc.sync.dma_start(out=b2_s[:], in_=b2_in[:])
            ident = constp.tile([P, P], F32)
            make_identity(nc, ident[:])
            sidx_s = sidxp.tile([P, SW], I32)
            nc.sync.dma_start(out=sidx_s[:], in_=sidx_in[:])

            sent_t = constp.tile([1, DEXT], F32)
            nc.vector.memset(sent_t[:], 0.0)
            nc.vector.memset(sent_t[:1, DH : DH + 1], SENT_EL)

            xt_s = xtp.tile([DH, NOWN], F32)
            nc.vector.memset(xt_s[:], 0.0)

            def feat_phase(layer):
                tbl = table1 if layer == 1 else table2
                nc.sync.dma_start(out=tbl[SENT : SENT + 1, :], in_=sent_t[:])
                if layer == 1:
                    gstarts = [(gb, min(FEATG, NBT - gb))
                               for gb in range(0, NBT, FEATG)]
                else:
                    gstarts = [
                        (cb * NBC + j0, min(FEATG, NBC - j0))
                        for cb in range(NCORES)
                        for j0 in range(0, NBC, FEATG)
                    ]
                for gb, gn in gstarts:
                    if layer == 1:
                        xtile = featin.tile([DIN, FEATG * P], F32, tag="htile")
                        nc.sync.dma_start(
                            out=xtile[:, : gn * P],
                            in_=ht_in[:, gb * P : (gb + gn) * P],
                        )
                        wmat = w1_s
                    else:
                        xtile = featin.tile([DH, FEATG * P], F32, tag="xtile")
                        cb, jb = gb // NBC, gb % NBC
                        nc.sync.dma_start(
                            out=xtile[:, : gn * P],
                            in_=xt_full[cb, :, jb * P : (jb + gn) * P],
                        )
                        wmat = w2_s
                    fout = featout.tile([P, FEATG * DEXT], F32, tag="fout")
                    for j in range(gn):
                        ps = psum.tile([P, DEXT], F32)
                        nc.tensor.matmul(
                            ps[:], xtile[:, j * P : (j + 1) * P], wmat[:]
                        )
                        nc.scalar.activation(
                            out=fout[:, j * DEXT : (j + 1) * DEXT],
                            in_=ps[:],
                            func=mybir.ActivationFunctionType.Copy,
                        )
                    nc.sync.dma_start(
                        out=tbl[gb * P : (gb + gn) * P, :].rearrange(
                            "(j p) d -> p j d", p=P
                        ),
                        in_=fout[:].rearrange("p (j d) -> p j d", d=DEXT)[
                            :, :gn, :
                        ],
                    )

            def agg_phase(layer):
                tbl = table1 if layer == 1 else table2
                for (i0, i1, c0, ncols) in groups:
                    g = gathp.tile([P, GC * DEXT], F32, tag="g")
                    for k in range(ncols):
                        nc.gpsimd.indirect_dma_start(
                            out=g[:, k * DEXT : (k + 1) * DEXT],
                            out_offset=None,
                            in_=tbl[:, :],
                            in_offset=bass.IndirectOffsetOnAxis(
                                ap=sidx_s[:, c0 + k : c0 + k + 1], axis=0
                            ),
                        )
                    for i in range(i0, i1):
                        K = int(K_sh[i])
                        base = int(cstart[i] - c0) * DEXT
                        blk = g[:, base : base + (1 + K) * DEXT].rearrange(
                            "p (k d) -> p k d", d=DEXT
                        )
                        er_col = blk[:, 0, DH + 1 : DH + 2]
                        el_mat = blk[:, 1 : 1 + K, DH]
                        feat3 = blk[:, 1 : 1 + K, 0:DH]

                        e0 = aggp.tile([P, K], F32, tag="e0")
                        nc.scalar.activation(
                            out=e0[:],
                            in_=el_mat,
                            func=mybir.ActivationFunctionType.Identity,
                            bias=er_col,
                        )
                        e1 = aggp.tile([P, K], F32, tag="e1")
                        nc.vector.tensor_scalar_mul(e1[:], e0[:], NEG)
                        et = aggp.tile([P, K], F32, tag="et")
                        nc.vector.tensor_tensor(
                            out=et[:], in0=e0[:], in1=e1[:],
                            op=mybir.AluOpType.max,
                        )
                        negm = smallp.tile([P, 1], F32, tag="negm")
                        nc.vector.tensor_reduce(
                            out=negm[:], in_=et[:], op=mybir.AluOpType.max,
                            axis=mybir.AxisListType.X, negate=True,
                        )
                        ee = aggp.tile([P, K], F32, tag="ee")
                        denom = smallp.tile([P, 1], F32, tag="denom")
                        nc.scalar.activation(
                            out=ee[:], in_=et[:],
                            func=mybir.ActivationFunctionType.Exp,
                            bias=negm[:], accum_out=denom[:],
                        )
                        rec = smallp.tile([P, 1], F32, tag="rec")
                        nc.vector.reciprocal(rec[:], denom[:])

                        prod = aggp.tile([P, K * DH], F32, tag="prod")
                        nc.vector.tensor_tensor(
                            out=prod[:].rearrange("p (k d) -> p k d", d=DH),
                            in0=feat3,
                            in1=ee[:].unsqueeze(2).to_broadcast([P, K, DH]),
                            op=mybir.AluOpType.mult,
                        )
                        acc = smallp.tile([P, DH], F32, tag="acc")
                        nc.vector.tensor_reduce(
                            out=acc[:],
                            in_=prod[:].rearrange("p (k d) -> p d k", d=DH),
                            op=mybir.AluOpType.add,
                            axis=mybir.AxisListType.X,
                        )
                        scaled = smallp.tile([P, DH], F32, tag="scaled")
                        nc.scalar.activation(
                            out=scaled[:], in_=acc[:],
                            func=mybir.ActivationFunctionType.Copy,
                            scale=rec[:],
                        )
                        if layer == 1:
                            pst = psum_t.tile([DH, P], F32)
                            nc.tensor.transpose(pst[:], scaled[:], ident[:])
                            nc.scalar.activation(
                                out=xt_s[:, i * P : (i + 1) * P],
                                in_=pst[:],
                                func=mybir.ActivationFunctionType.Identity,
                                bias=b1_s[:],
                            )
                        else:
                            outf = smallp.tile([P, DH], F32, tag="outf")
                            nc.vector.tensor_tensor(
                                out=outf[:], in0=scaled[:], in1=b2_s[:],
                                op=mybir.AluOpType.add,
                            )
                            nc.sync.dma_start(
                                out=out_ext[i * P : (i + 1) * P, :], in_=outf[:]
                            )

            feat_phase(1)
            agg_phase(1)

            nc.sync.dma_start(out=xt_own[:, :], in_=xt_s[:])
            nc.gpsimd.collective_compute(
                "AllGather",
                mybir.AluOpType.bypass,
                replica_groups=[list(range(NCORES))],
                ins=[xt_own[:, :].opt()],
                outs=[xt_full[:, :, :].opt()],
            )

            feat_phase(2)
            agg_phase(2)

    nc.finalize()
    return nc


_CACHE = {}
TRACE = False
RUN_KWARGS = {}
LAST_RESULT = None


def _get_compiled(key, plan):
    if key not in _CACHE:
        _CACHE[key] = _build_nc(plan)
    return _CACHE[key]


def _make_wext(W, al, ar):
    We = W.astype(np.float64)
    wal = We.T @ al.astype(np.float64)
    war = We.T @ ar.astype(np.float64)
    return np.concatenate(
        [We.T, wal[:, None], war[:, None]], axis=1
    ).astype(np.float32)


def _make_in_maps(plan, inputs):
    features = np.ascontiguousarray(np.asarray(inputs["features"], np.float32))
    W1 = np.asarray(inputs["W1"], np.float32)
    al1 = np.asarray(inputs["al1"], np.float32)
    ar1 = np.asarray(inputs["ar1"], np.float32)
    b1 = np.asarray(inputs["b1"], np.float32)
    W2 = np.asarray(inputs["W2"], np.float32)
    al2 = np.asarray(inputs["al2"], np.float32)
    ar2 = np.asarray(inputs["ar2"], np.float32)
    b2 = np.asarray(inputs["b2"], np.float32)

    NPOS = plan["NPOS"]
    q_of_s, vmask, node_at_s = plan["q_of_s"], plan["vmask"], plan["node_at_s"]
    hperm = np.zeros((NPOS, DIN), np.float32)
    hperm[q_of_s[vmask]] = features[node_at_s[vmask]]
    ht = np.ascontiguousarray(hperm.T)

    w1ext = _make_wext(W1, al1, ar1)
    w2ext = _make_wext(W2, al2, ar2)
    b1c = np.ascontiguousarray(b1[:, None])
    b2t = np.ascontiguousarray(np.broadcast_to(b2[None, :], (P, DH)))

    in_maps = []
    for c in range(NCORES):
        in_maps.append(
            {
                "ht": ht,
                "sidx": np.ascontiguousarray(plan["sidx"][c]),
                "w1ext": w1ext,
                "w2ext": w2ext,
                "b1c": b1c,
                "b2t": b2t,
            }
        )
    return in_maps


def kernel(**inputs):
    features = np.asarray(inputs["features"], np.float32)
    src = np.asarray(inputs["src"]).astype(np.int64)
    dst = np.asarray(inputs["dst"]).astype(np.int64)

    n_nodes = features.shape[0]
    plan = _plan(src, dst, n_nodes)

    nc = _get_compiled((n_nodes, src.shape[0], plan["SW"]), plan)
    in_maps = _make_in_maps(plan, inputs)

    res = run_bass_kernel_spmd(
        nc, in_maps, core_ids=list(range(NCORES)), trace=TRACE, **RUN_KWARGS
    )
    global LAST_RESULT
    LAST_RESULT = res
    out_cat = np.concatenate([r["out"] for r in res.results], axis=0)

    node_at_q = plan["node_at_q"]
    outv = np.zeros((n_nodes, DH), np.float32)
    m = node_at_q >= 0
    outv[node_at_q[m]] = out_cat[m]
    return outv

